# revision 1
# baseline (speedup 1.0000x reference)
"""Neural CDE Trainium2 kernel.

Strategy: pure data parallelism over batch B=128 -> 8 cores x 16 rows.
Per core, the T-1=1023-step RK4 scan runs as a fully unrolled sequential
chain. Layout: activations [feature_on_partition, batch_on_free].

Math notes:
  - softplus = Ln(Exp(z)+1) using the natural_log_exp_and_others ACT table
    (the only table covering every transcendental used in the loop: Exp,
    Ln, Relu, Identity). Layer biases ride the ACT bias slot ([P,1] AP).
  - tanh(v) = 1 - 2/(1+exp(2v)): Exp on ACT, min/+1 dual-op tensor_scalar,
    reciprocal_approx_fast on DVE.
  - einsum('bhd,bd->bh', tanh(V), dX) with tanh expanded:
        k*a = a*S - 2a * G.T @ (r * Z)
    S[b] = sum_d dX[b,d] (all-alpha [8,64] matmul), Z[p,b] = dX[b, p//16]
    (selector matmul), G one-hot selectors with -2a baked in. fw2 rows are
    permuted so chunk c / partition p hold (h = 16c + p%16, d = p//16).
  - fb2 enters PSUM first via a rank-4 constant matmul (has_written rule).
  - RK4 combination tracked with affine_then_add ops off the chain.

Sync-wait constraint: this walrus build allows a single on_wait per
Matmult, so ALL constants ship in ONE packed DRAM tensor (one DMA queue =
one semaphore) and the per-step dX slice is staged through a DVE copy so
matmuls only ever wait on one producer engine.
"""

import numpy as np

B, T, D, H, W = 128, 1024, 8, 64, 128
NCORES = 8
BS = B // NCORES          # 16 batch rows per core
NSTEPS_FULL = T - 1       # 1023

_CJ = (1.0 / 3.0, 2.0 / 3.0, 1.0 / 3.0, 1.0)  # u_j / alpha_j for y' accum
_SROW = (0, 1, 1, 2)                   # dX variant per stage
_AVARIANT = (0, 0, 1, 2)               # alpha variant {0.5, 1.0, 1/6}
_AVALS = (0.5, 1.0, 1.0 / 6.0)

# wconst free-dim layout: name -> (partitions, free_offset, free_len)
_L = {}
_off = 0
for _name, _p, _f in [
    ("fw0p", H, W), ("fw1p", W, W), ("fw2p", W, 512),
    ("gneg", 128, 3 * 4 * H), ("ebc", D, 128), ("onesa", D, 3 * H),
    ("b3l", 4, 128), ("b3r", 4, 4 * BS),
    ("iw0p", D, W), ("iw1p", W, W), ("iw2p", W, H),
    ("x0T", D, BS), ("lwT", H, 1),
    ("ib0", W, 1), ("ib1", W, 1), ("ib2", H, 1),
    ("fb0", W, 1), ("fb1", W, 1), ("lbneg", 1, 1),
]:
    _L[_name] = (_p, _off, _f)
    _off += _f
WCONST_F = _off


def _hd_orig(c, p):
    h = 16 * c + (p % 16)
    d = p // 16
    return h * D + d


def build_bass(nsteps):
    import concourse.bass as bass
    import concourse.bacc as bacc
    import concourse.mybir as mybir
    from concourse import tile

    f32 = mybir.dt.float32
    AF = mybir.ActivationFunctionType
    ALU = mybir.AluOpType

    # Bacc (not Bass): its compile() runs move_matmul_waits_to_ldweights +
    # generate_event_semaphores, which legalize multi-wait instructions for
    # walrus (1 on_wait per instruction on TRN2).
    nc = bacc.Bacc(None)

    wc_d = nc.declare_dram_parameter("wconst", [128, WCONST_F], f32, isOutput=False)
    dxt_d = [
        nc.declare_dram_parameter(f"dxt{s}", [D, nsteps * BS], f32, isOutput=False)
        for s in range(3)
    ]
    out_d = nc.declare_dram_parameter("out", [1, BS], f32, isOutput=True)

    with tile.TileContext(nc) as tc:
        with (
            tc.tile_pool(name="const", bufs=1) as cpool,
            tc.tile_pool(name="ybase", bufs=1) as ypool,
            tc.tile_pool(name="acc", bufs=1) as apool,
            tc.tile_pool(name="ycur", bufs=2) as ycpool,
            tc.tile_pool(name="work16", bufs=2) as w16,
            tc.tile_pool(name="work64", bufs=2) as w64,
            tc.tile_pool(name="ps_zb", bufs=1, space="PSUM") as ps_zb,
            tc.tile_pool(name="ps_korr", bufs=1, space="PSUM") as ps_korr,
            tc.tile_pool(name="ps_p1", bufs=2, space="PSUM") as ps_p1,
            tc.tile_pool(name="ps_p2", bufs=1, space="PSUM") as ps_p2,
            tc.tile_pool(name="ps_u", bufs=1, space="PSUM") as ps_u,
            tc.tile_pool(name="ps_p3", bufs=1, space="PSUM") as ps_p3,
            tc.tile_pool(name="ps_kneg", bufs=1, space="PSUM") as ps_kneg,
        ):
            wc = cpool.tile([128, WCONST_F], f32, tag="wconst")
            nc.sync.dma_start(wc[:], wc_d[:])
            dxt = []
            for s in range(3):
                dt_ = cpool.tile([D, nsteps * BS], f32, tag=f"dxt{s}")
                nc.sync.dma_start(dt_[:], dxt_d[s][:])
                dxt.append(dt_)

            def C(name):
                p, o, f = _L[name]
                return wc[0:p, o : o + f]

            # Warm each non-PE engine's vector clock on the const DMAs so
            # later ops never carry a DMA wait alongside an engine wait
            # (single on_wait slot per instruction in this walrus build).
            warm = w16.tile([1, 4], f32, tag="warm")
            nc.scalar.activation(warm[0:1, 0:1], wc[0:1, 0:1], AF.Copy)
            nc.vector.tensor_copy(warm[0:1, 1:2], wc[0:1, 0:1])
            for s in range(3):
                nc.vector.tensor_copy(warm[0:1, 1:2], dxt[s][0:1, 0:1])

            # ---- y0 = init_mlp(x0) ----
            y = ypool.tile([H, BS], f32, tag="y")
            A = apool.tile([H, BS], f32, tag="A")

            pi = ps_p1.tile([W, BS], f32, tag="p1")
            nc.tensor.matmul(pi[:], C("iw0p"), C("x0T"), start=True, stop=True)
            h1 = w16.tile([W, BS], f32, tag="s")
            nc.scalar.activation(h1[:], pi[:], AF.Relu, bias=C("ib0"))
            pi2 = ps_p2.tile([W, BS], f32, tag="p2")
            nc.tensor.matmul(pi2[:], C("iw1p"), h1[:], start=True, stop=True)
            h2 = w16.tile([W, BS], f32, tag="s")
            nc.scalar.activation(h2[:], pi2[:], AF.Relu, bias=C("ib1"))
            pk = ps_kneg.tile([H, BS], f32, tag="kneg")
            nc.tensor.matmul(pk[:], C("iw2p"), h2[:], start=True, stop=True)
            nc.scalar.activation(y[:], pk[:], AF.Identity, bias=C("ib2"))

            # ---- the scan ----
            for t in range(nsteps):
                ycur = y
                for j in range(4):
                    s = _SROW[j]
                    av = _AVARIANT[j]
                    cj = _CJ[j]
                    tb = t * BS

                    # off-chain: stage dX slice via DVE, then Z / korr mms
                    dxs = w16.tile([D, BS], f32, tag="dxs")
                    nc.vector.tensor_copy(dxs[:], dxt[s][:, tb : tb + BS])

                    zb_ps = ps_zb.tile([128, BS], f32, tag="zb")
                    nc.tensor.matmul(zb_ps[:], C("ebc"), dxs[:], start=True, stop=True)
                    zb = w16.tile([128, BS], f32, tag="zb_sb")
                    nc.vector.tensor_copy(zb[:], zb_ps[:])

                    korr = ps_korr.tile([H, BS], f32, tag="korr")
                    oa = C("onesa")
                    nc.tensor.matmul(
                        korr[:], oa[:, av * H : (av + 1) * H], dxs[:],
                        start=True, stop=True,
                    )

                    # chain: MLP layer 1
                    p1 = ps_p1.tile([W, BS], f32, tag="p1")
                    nc.tensor.matmul(p1[:], C("fw0p"), ycur[:], start=True, stop=True)
                    u1 = ps_u.tile([W, BS], f32, tag="u")
                    nc.scalar.activation(u1[:], p1[:], AF.Exp, bias=C("fb0"))
                    s1 = w16.tile([W, BS], f32, tag="s")
                    nc.scalar.activation(s1[:], u1[:], AF.Ln, bias=1.0)

                    # chain: MLP layer 2
                    p2 = ps_p2.tile([W, BS], f32, tag="p2")
                    nc.tensor.matmul(p2[:], C("fw1p"), s1[:], start=True, stop=True)
                    u2 = ps_u.tile([W, BS], f32, tag="u")
                    nc.scalar.activation(u2[:], p2[:], AF.Exp, bias=C("fb1"))
                    s2 = w16.tile([W, BS], f32, tag="s")
                    nc.scalar.activation(s2[:], u2[:], AF.Ln, bias=1.0)

                    # chain: MLP layer 3 (4 chunks) + fb2 rank-4 bias mm
                    p3 = ps_p3.tile([128, 4 * BS], f32, tag="p3")
                    nc.tensor.matmul(p3[:], C("b3l"), C("b3r"), start=True, stop=False)
                    fw2p = C("fw2p")
                    for c in range(4):
                        nc.tensor.matmul(
                            p3[:, c * BS : (c + 1) * BS],
                            fw2p[:, c * 128 : (c + 1) * 128],
                            s2[:],
                            start=False, stop=(c == 3),
                        )

                    # chain: tanh pieces
                    texp = w64.tile([128, 4 * BS], f32, tag="texp")
                    nc.scalar.activation(texp[:], p3[:], AF.Exp, scale=2.0)
                    den = w64.tile([128, 4 * BS], f32, tag="den")
                    nc.vector.tensor_scalar(
                        den[:], texp[:], 1.0e30, 1.0, ALU.min, ALU.add
                    )
                    r = w64.tile([128, 4 * BS], f32, tag="r")
                    nc.vector.reciprocal_approx_fast(r[:], den[:])

                    # chain: rZ = r * Z  (Z broadcast along the 4 chunks)
                    rZ = w64.tile([128, 4, BS], f32, tag="rZ")
                    zb_b = zb[:, :]
                    zb_b = bass.AP(
                        zb_b.tensor, zb_b.offset,
                        [zb_b.ap[0], [0, 4], zb_b.ap[1]],
                    )
                    r3 = r[:, :]
                    r3 = bass.AP(
                        r3.tensor, r3.offset,
                        [r3.ap[0], [BS, 4], [1, BS]],
                    )
                    nc.vector.tensor_tensor(rZ[:], r3, zb_b, ALU.mult)

                    # chain: kneg = G(-2a).T @ rZ (4 accumulating mms)
                    kneg = ps_kneg.tile([H, BS], f32, tag="kneg")
                    gn = C("gneg")
                    for c in range(4):
                        nc.tensor.matmul(
                            kneg[:],
                            gn[:, (av * 4 + c) * H : (av * 4 + c + 1) * H],
                            rZ[:, c, :],
                            start=(c == 0), stop=(c == 3),
                        )

                    # bookkeeping (off chain) + next-stage input (chain)
                    if j == 0:
                        nc.vector.affine_then_add(A[:], korr[:], y[:], cj, 0.0)
                    else:
                        nc.vector.affine_then_add(A[:], korr[:], A[:], cj, 0.0)
                    nc.vector.affine_then_add(A[:], kneg[:], A[:], cj, 0.0)

                    if j < 3:
                        yk = w16.tile([H, BS], f32, tag="yk")
                        nc.vector.tensor_tensor(yk[:], y[:], korr[:], ALU.add)
                        ynext = ycpool.tile([H, BS], f32, tag="ycur")
                        nc.vector.tensor_tensor(ynext[:], yk[:], kneg[:], ALU.add)
                        ycur = ynext
                    else:
                        # A now holds y + sum_j u_j k_j = y_{t+1}
                        nc.vector.tensor_copy(y[:], A[:])

            # ---- readout: sigmoid(lw @ y + lb) ----
            pr = ps_korr.tile([1, BS], f32, tag="korr")
            nc.tensor.matmul(pr[:], C("lwT"), y[:], start=True, stop=True)
            er = w16.tile([1, BS], f32, tag="er")
            nc.scalar.activation(er[:], pr[:], AF.Exp, bias=C("lbneg"), scale=-1.0)
            dr = w16.tile([1, BS], f32, tag="dr")
            nc.vector.tensor_scalar_add(dr[:], er[:], 1.0)
            rr = w16.tile([1, BS], f32, tag="rr")
            nc.vector.reciprocal(rr[:], dr[:])
            nc.sync.dma_start(out_d[:], rr[:])

    nc.compile()
    return nc


def prep_inputs(ts, coeff_d, coeff_c, coeff_b, coeff_a,
                iw0, ib0, iw1, ib1, iw2, ib2,
                fw0, fb0, fw1, fb1, fw2, fb2, lw, lb, nsteps=NSTEPS_FULL):
    """Build per-core input maps (host-side numpy prep)."""
    f = np.float32
    cd = np.asarray(coeff_d, f)[:, :nsteps, :]
    cc = np.asarray(coeff_c, f)[:, :nsteps, :]
    cb = np.asarray(coeff_b, f)[:, :nsteps, :]
    ca = np.asarray(coeff_a, f)

    dX1 = cb
    dX23 = 0.75 * cd + cc + cb
    dX4 = 3.0 * cd + 2.0 * cc + cb

    fw2 = np.asarray(fw2, f)
    fb2 = np.asarray(fb2, f)

    def fill(wc, name, arr):
        p, o, fl = _L[name]
        assert arr.shape == (p, fl), (name, arr.shape, (p, fl))
        wc[0:p, o : o + fl] = arr

    wc0 = np.zeros((128, WCONST_F), f)
    fill(wc0, "fw0p", np.ascontiguousarray(np.asarray(fw0, f).T))
    fill(wc0, "fw1p", np.ascontiguousarray(np.asarray(fw1, f).T))
    fw2p = np.zeros((W, 512), f)
    b3l = np.zeros((4, 128), f)
    for c in range(4):
        for p in range(128):
            hd = _hd_orig(c, p)
            fw2p[:, c * 128 + p] = fw2[hd, :]
            b3l[c, p] = fb2[hd]
    fill(wc0, "fw2p", fw2p)
    fill(wc0, "b3l", b3l)
    b3r = np.zeros((4, 4 * BS), f)
    for c in range(4):
        b3r[c, c * BS : (c + 1) * BS] = 1.0
    fill(wc0, "b3r", b3r)
    gneg = np.zeros((128, 3 * 4 * H), f)
    for ai, aval in enumerate(_AVALS):
        for c in range(4):
            for p in range(128):
                h = 16 * c + (p % 16)
                gneg[p, (ai * 4 + c) * H + h] = -2.0 * aval
    fill(wc0, "gneg", gneg)
    onesa = np.zeros((D, 3 * H), f)
    for ai, aval in enumerate(_AVALS):
        onesa[:, ai * H : (ai + 1) * H] = aval
    fill(wc0, "onesa", onesa)
    ebc = np.zeros((D, 128), f)
    for p in range(128):
        ebc[p // 16, p] = 1.0
    fill(wc0, "ebc", ebc)
    fill(wc0, "iw0p", np.ascontiguousarray(np.asarray(iw0, f).T))
    fill(wc0, "iw1p", np.ascontiguousarray(np.asarray(iw1, f).T))
    fill(wc0, "iw2p", np.ascontiguousarray(np.asarray(iw2, f).T))
    fill(wc0, "lwT", np.ascontiguousarray(np.asarray(lw, f).reshape(1, H).T))
    fill(wc0, "ib0", np.asarray(ib0, f)[:, None])
    fill(wc0, "ib1", np.asarray(ib1, f)[:, None])
    fill(wc0, "ib2", np.asarray(ib2, f)[:, None])
    fill(wc0, "fb0", np.asarray(fb0, f)[:, None])
    fill(wc0, "fb1", np.asarray(fb1, f)[:, None])
    fill(wc0, "lbneg", -np.asarray(lb, f).reshape(1, 1))

    in_maps = []
    for i in range(NCORES):
        sl = slice(i * BS, (i + 1) * BS)
        wc = wc0.copy()
        fill(wc, "x0T", np.ascontiguousarray(ca[sl, 0, :].T))
        m = {"wconst": wc}
        for name, arr in (("dxt0", dX1), ("dxt1", dX23), ("dxt2", dX4)):
            m[name] = np.ascontiguousarray(
                arr[sl].transpose(2, 1, 0).reshape(D, -1)
            )
        in_maps.append(m)
    return in_maps


_CACHE = {}


def _get_nc(nsteps):
    if nsteps not in _CACHE:
        _CACHE[nsteps] = build_bass(nsteps)
    return _CACHE[nsteps]


def kernel(**inputs):
    from concourse.bass_utils import run_bass_kernel_spmd

    nsteps = NSTEPS_FULL
    in_maps = prep_inputs(nsteps=nsteps, **inputs)
    nc = _get_nc(nsteps)
    res = run_bass_kernel_spmd(nc, in_maps, list(range(NCORES)))
    outs = [res.results[i]["out"].reshape(BS) for i in range(NCORES)]
    return np.concatenate(outs, axis=0).astype(np.float32)



# revision 7
# speedup vs baseline: 1.5428x; 1.5428x over previous
"""Neural CDE Trainium2 kernel.

Strategy: pure data parallelism over batch B=128 -> 8 cores x 16 rows.
Per core, the T-1=1023-step RK4 scan runs inside a For_i hardware loop
(U=3 steps per body, 341 iterations) so the whole program fits in each
engine's IRAM block -- the fully unrolled variant (~90k instructions)
pays NEFF-size, icache and compile costs ~50x larger.

Layout: activations [feature_on_partition, batch_on_free].

Math notes (ts = arange -> h = 1, stage times s in {0, .5, .5, 1}):
  - dX variants precomputed on host: dX1 = b, dX23 = .75 d + c + b,
    dX4 = 3 d + 2 c + b, stored [D=8, T*BS] per variant.
  - softplus = Ln(Exp(z)+1) via the natural_log_exp_and_others ACT table,
    preloaded ONCE via a manual InstLoadActFuncSet (set id 6). Without the
    preload, bacc's table pass alternates exp_and_others/natural_log
    (~16 reloads/step at 1.3us each -- dominates everything).
  - tanh(v) = 1 - 2/(1+exp(2v)): Exp on ACT, min/+1 dual-op tensor_scalar,
    reciprocal_approx_fast on DVE.
  - einsum('bhd,bd->bh', tanh(V), dX), tanh expanded, folded into ONE
    accumulating PSUM group per stage:
        k*a = a*S - 2a * G.T @ (r * Z)
    seeded by matmul(onesa[a-variant], dxs) (= a*S broadcast over h), then
    4 accumulating selector matmuls G(-2a).T @ (r*Z). fw2 rows are permuted
    so chunk c / partition p hold (h = 16c + p%16, d = p//16).
  - Z[p, b] = dX[p//16, b] arrives pre-broadcast by a 0-stride DMA read of
    the dxt rows (no ebc selector matmul / PSUM round-trip needed).
  - fb2 enters PSUM first via a rank-4 constant matmul (has_written rule).
  - y_{t+1} accumulated off-chain: A = y + sum_j cj * k~_j via
    affine_then_add; the j=3 affine writes y directly.
"""

import numpy as np

B, T, D, H, W = 128, 1024, 8, 64, 128
NCORES = 8
BS = B // NCORES          # 16 batch rows per core
NSTEPS_FULL = T - 1       # 1023
UNROLL = 3                # steps per For_i body; 1023 = 3 * 341

_CJ = (1.0 / 3.0, 2.0 / 3.0, 1.0 / 3.0, 1.0)  # u_j / alpha_j for y' accum
_SROW = (0, 1, 1, 2)                   # dX variant per stage
_AVARIANT = (0, 0, 1, 2)               # alpha variant {0.5, 1.0, 1/6}
_AVALS = (0.5, 1.0, 1.0 / 6.0)

# wconst free-dim layout: name -> (partitions, free_offset, free_len)
_L = {}
_off = 0
for _name, _p, _f in [
    ("fw0p", H, W), ("fw1p", W, W), ("fw2p", W, 512),
    ("gneg", 128, 3 * 4 * H), ("onesa", D, 3 * H),
    ("ww", 128, 2 * 4 * W), ("fsa", D, 2 * W),
    ("b3l", 4, 128), ("b3r", 4, 4 * BS),
    ("iw0p", D, W), ("iw1p", W, W), ("iw2p", W, H),
    ("x0T", D, BS), ("lwT", H, 1),
    ("ib0", W, 1), ("ib1", W, 1), ("ib2", H, 1),
    ("fb0", W, 1), ("fb1", W, 1), ("lbneg", 1, 1),
]:
    _L[_name] = (_p, _off, _f)
    _off += _f
WCONST_F = _off


def _hd_orig(c, p):
    h = 16 * c + (p % 16)
    d = p // 16
    return h * D + d


def build_bass(nsteps, hw_loop=True):
    import concourse.bass as bass
    import concourse.bacc as bacc
    import concourse.mybir as mybir
    from concourse import tile

    f32 = mybir.dt.float32
    AF = mybir.ActivationFunctionType
    ALU = mybir.AluOpType

    assert nsteps % UNROLL == 0, (nsteps, UNROLL)
    niters = nsteps // UNROLL
    UB = UNROLL * BS

    # Bacc (not Bass): its compile() runs move_matmul_waits_to_ldweights +
    # generate_event_semaphores, which legalize multi-wait instructions for
    # walrus (1 on_wait per instruction on TRN2).
    nc = bacc.Bacc(None)

    wc_d = nc.declare_dram_parameter("wconst", [128, WCONST_F], f32, isOutput=False)
    dxt_d = [
        nc.declare_dram_parameter(f"dxt{s}", [D, nsteps * BS], f32, isOutput=False)
        for s in range(3)
    ]
    out_d = nc.declare_dram_parameter("out", [1, BS], f32, isOutput=True)

    def z_src_ap(s, off):
        # [128, UB] read of dxt_d[s] where partition p sources row p//16:
        # dims (d:8, repeat:16 stride 0, col:UB).
        base = dxt_d[s][:, bass.ds(off, UB) if not isinstance(off, int)
                        else slice(off, off + UB)]
        return bass.AP(base.tensor, base.offset, [base.ap[0], [0, BS], base.ap[1]])

    with tile.TileContext(nc) as tc:
        with (
            tc.tile_pool(name="const", bufs=1) as cpool,
            tc.tile_pool(name="ybase", bufs=1) as ypool,
            tc.tile_pool(name="acc", bufs=1) as apool,
            tc.tile_pool(name="zblk", bufs=2) as zpool,
            tc.tile_pool(name="ycur", bufs=2) as ycpool,
            tc.tile_pool(name="work16", bufs=2) as w16,
            tc.tile_pool(name="work64", bufs=2) as w64,
            tc.tile_pool(name="ps_p1", bufs=2, space="PSUM") as ps_p1,
            tc.tile_pool(name="ps_p2", bufs=2, space="PSUM") as ps_p2,
            tc.tile_pool(name="ps_p3", bufs=2, space="PSUM") as ps_p3,
            tc.tile_pool(name="ps_kneg", bufs=2, space="PSUM") as ps_kneg,
        ):
            wc = cpool.tile([128, WCONST_F], f32, tag="wconst")
            nc.sync.dma_start(wc[:], wc_d[:])

            def C(name):
                p, o, f = _L[name]
                return wc[0:p, o : o + f]

            # Preload the natural_log_exp_and_others ACT table set (id 6:
            # Exp, Ln, Relu, Identity, Copy) once; the bacc fixpoint then
            # proves every activation resident and inserts no further loads.
            ld = mybir.InstLoadActFuncSet(
                name=nc.get_next_instruction_name(), ins=[], outs=[]
            )
            ld.act_func_set_id = 6
            nc.scalar.add_instruction(ld)

            # Warm each non-PE engine's vector clock on the const DMA so
            # later ops never carry a DMA wait alongside an engine wait
            # (single on_wait slot per instruction in this walrus build).
            warm = w16.tile([1, 4], f32, tag="warm")
            nc.scalar.activation(warm[0:1, 0:1], wc[0:1, 0:1], AF.Copy)
            nc.vector.tensor_copy(warm[0:1, 1:2], wc[0:1, 0:1])

            # ---- y0 = init_mlp(x0) ----
            y = ypool.tile([H, BS], f32, tag="y")
            A = apool.tile([H, BS], f32, tag="A")

            pi = ps_p1.tile([W, BS], f32, tag="p1")
            nc.tensor.matmul(pi[:], C("iw0p"), C("x0T"), start=True, stop=True)
            h1 = w16.tile([W, BS], f32, tag="s")
            nc.scalar.activation(h1[:], pi[:], AF.Relu, bias=C("ib0"))
            pi2 = ps_p2.tile([W, BS], f32, tag="p2")
            nc.tensor.matmul(pi2[:], C("iw1p"), h1[:], start=True, stop=True)
            h2 = w16.tile([W, BS], f32, tag="s")
            nc.scalar.activation(h2[:], pi2[:], AF.Relu, bias=C("ib1"))
            pk = ps_kneg.tile([H, BS], f32, tag="kneg")
            nc.tensor.matmul(pk[:], C("iw2p"), h2[:], start=True, stop=True)
            nc.scalar.activation(y[:], pk[:], AF.Identity, bias=C("ib2"))

            # ---- the scan: For_i over blocks of UNROLL steps ----
            def block_body(iv):
                # Stream this block's dX rows ([8, UB] for the a*S seeds)
                # and the 0-stride-expanded Z form ([128, UB]) per variant.
                dxs_t, z_t = [], []
                for s in range(3):
                    dt_ = zpool.tile([D, UB], f32, tag=f"dxs{s}")
                    if isinstance(iv, int):
                        nc.sync.dma_start(dt_[:], dxt_d[s][:, iv : iv + UB])
                    else:
                        nc.sync.dma_start(dt_[:], dxt_d[s][:, bass.ds(iv, UB)])
                    dxs_t.append(dt_)
                    zt = zpool.tile([128, UB], f32, tag=f"z{s}")
                    nc.sync.dma_start(zt[:], z_src_ap(s, iv))
                    z_t.append(zt)

                for u in range(UNROLL):
                    p1_next = None
                    for j in range(4):
                        s = _SROW[j]
                        av = _AVARIANT[j]
                        cj = _CJ[j]
                        ub0 = u * BS
                        dxs = dxs_t[s][:, ub0 : ub0 + BS]

                        # k~ PSUM group seed: a*S broadcast over h (off-chain)
                        kneg = ps_kneg.tile([H, BS], f32, tag="kneg")
                        oa = C("onesa")
                        nc.tensor.matmul(
                            kneg[:], oa[:, av * H : (av + 1) * H], dxs,
                            start=True, stop=False,
                        )

                        # p1_j: stage 0 computes fw0 @ y directly; stages 1-3
                        # use the PSUM group seeded in the previous stage
                        # (fw0 @ y + a_j S fsum, closed by the WW mms below).
                        if j == 0:
                            p1 = ps_p1.tile([W, BS], f32, tag="p1")
                            nc.tensor.matmul(p1[:], C("fw0p"), y[:], start=True, stop=True)
                        else:
                            p1 = p1_next

                        # seed next stage's p1 group: fw0@y + a_j S fsum
                        # (y + k~_j is never materialized; fw0 @ k~_j arrives
                        # via the WW matmuls at this stage's chain tail).
                        if j < 3:
                            p1_next = ps_p1.tile([W, BS], f32, tag="p1")
                            nc.tensor.matmul(
                                p1_next[:], C("fw0p"), y[:], start=True, stop=False
                            )
                            fsa = C("fsa")
                            nc.tensor.matmul(
                                p1_next[:], fsa[:, av * W : (av + 1) * W], dxs,
                                start=False, stop=False,
                            )

                        # chain: softplus layer 1
                        u1 = w16.tile([W, BS], f32, tag="u")
                        nc.scalar.activation(u1[:], p1[:], AF.Exp, bias=C("fb0"))
                        s1 = w16.tile([W, BS], f32, tag="s")
                        nc.scalar.activation(s1[:], u1[:], AF.Ln, bias=1.0)

                        # chain: MLP layer 2
                        p2 = ps_p2.tile([W, BS], f32, tag="p2")
                        nc.tensor.matmul(p2[:], C("fw1p"), s1[:], start=True, stop=True)
                        u2 = w16.tile([W, BS], f32, tag="u")
                        nc.scalar.activation(u2[:], p2[:], AF.Exp, bias=C("fb1"))
                        s2 = w16.tile([W, BS], f32, tag="s")
                        nc.scalar.activation(s2[:], u2[:], AF.Ln, bias=1.0)

                        # chain: MLP layer 3 (4 chunks) + fb2 rank-4 bias mm
                        p3 = ps_p3.tile([128, 4 * BS], f32, tag="p3")
                        nc.tensor.matmul(p3[:], C("b3l"), C("b3r"), start=True, stop=False)
                        fw2p = C("fw2p")
                        for c in range(4):
                            nc.tensor.matmul(
                                p3[:, c * BS : (c + 1) * BS],
                                fw2p[:, c * 128 : (c + 1) * 128],
                                s2[:],
                                start=False, stop=(c == 3),
                            )

                        # chain: tanh pieces
                        texp = w64.tile([128, 4 * BS], f32, tag="texp")
                        nc.scalar.activation(texp[:], p3[:], AF.Exp, scale=2.0)
                        den = w64.tile([128, 4 * BS], f32, tag="den")
                        nc.vector.tensor_scalar(
                            den[:], texp[:], 1.0e30, 1.0, ALU.min, ALU.add
                        )
                        r = w64.tile([128, 4 * BS], f32, tag="r")
                        nc.vector.reciprocal_approx_fast(r[:], den[:])

                        # chain: rZ = r * Z  (Z broadcast along the 4 chunks)
                        rZ = w64.tile([128, 4, BS], f32, tag="rZ")
                        zb_b = z_t[s][:, ub0 : ub0 + BS]
                        zb_b = bass.AP(
                            zb_b.tensor, zb_b.offset,
                            [zb_b.ap[0], [0, 4], zb_b.ap[1]],
                        )
                        r3 = r[:, :]
                        r3 = bass.AP(
                            r3.tensor, r3.offset,
                            [r3.ap[0], [BS, 4], [1, BS]],
                        )
                        nc.vector.tensor_tensor(rZ[:], r3, zb_b, ALU.mult)

                        # chain tail: fw0 @ k~_j folded into next p1 group
                        if j < 3:
                            wwt = C("ww")
                            for c in range(4):
                                nc.tensor.matmul(
                                    p1_next[:],
                                    wwt[:, (av * 4 + c) * W : (av * 4 + c + 1) * W],
                                    rZ[:, c, :],
                                    start=False, stop=(c == 3),
                                )

                        # off chain: accumulate -2a G.T @ rZ onto the a*S seed
                        gn = C("gneg")
                        for c in range(4):
                            nc.tensor.matmul(
                                kneg[:],
                                gn[:, (av * 4 + c) * H : (av * 4 + c + 1) * H],
                                rZ[:, c, :],
                                start=False, stop=(c == 3),
                            )

                        # RK4 accumulator (off chain until the j=3 y write)
                        if j == 0:
                            nc.vector.affine_then_add(A[:], kneg[:], y[:], cj, 0.0)
                        elif j < 3:
                            nc.vector.affine_then_add(A[:], kneg[:], A[:], cj, 0.0)
                        else:
                            # y_{t+1} = A + cj * k~_3, written into y
                            nc.vector.affine_then_add(y[:], kneg[:], A[:], cj, 0.0)

            if hw_loop:
                with tc.For_i(0, nsteps * BS, UB) as iv:
                    block_body(iv)
            else:
                for it in range(niters):
                    block_body(it * UB)

            # ---- readout: sigmoid(lw @ y + lb) ----
            pr = ps_p2.tile([1, BS], f32, tag="p2")
            nc.tensor.matmul(pr[:], C("lwT"), y[:], start=True, stop=True)
            er = w16.tile([1, BS], f32, tag="er")
            nc.scalar.activation(er[:], pr[:], AF.Exp, bias=C("lbneg"), scale=-1.0)
            dr = w16.tile([1, BS], f32, tag="dr")
            nc.vector.tensor_scalar_add(dr[:], er[:], 1.0)
            rr = w16.tile([1, BS], f32, tag="rr")
            nc.vector.reciprocal(rr[:], dr[:])
            nc.sync.dma_start(out_d[:], rr[:])

    nc.compile()
    return nc


def prep_inputs(ts, coeff_d, coeff_c, coeff_b, coeff_a,
                iw0, ib0, iw1, ib1, iw2, ib2,
                fw0, fb0, fw1, fb1, fw2, fb2, lw, lb, nsteps=NSTEPS_FULL):
    """Build per-core input maps (host-side numpy prep)."""
    f = np.float32
    cd = np.asarray(coeff_d, f)[:, :nsteps, :]
    cc = np.asarray(coeff_c, f)[:, :nsteps, :]
    cb = np.asarray(coeff_b, f)[:, :nsteps, :]
    ca = np.asarray(coeff_a, f)

    dX1 = cb
    dX23 = 0.75 * cd + cc + cb
    dX4 = 3.0 * cd + 2.0 * cc + cb

    fw2 = np.asarray(fw2, f)
    fb2 = np.asarray(fb2, f)

    def fill(wc, name, arr):
        p, o, fl = _L[name]
        assert arr.shape == (p, fl), (name, arr.shape, (p, fl))
        wc[0:p, o : o + fl] = arr

    wc0 = np.zeros((128, WCONST_F), f)
    fill(wc0, "fw0p", np.ascontiguousarray(np.asarray(fw0, f).T))
    fill(wc0, "fw1p", np.ascontiguousarray(np.asarray(fw1, f).T))
    fw2p = np.zeros((W, 512), f)
    b3l = np.zeros((4, 128), f)
    for c in range(4):
        for p in range(128):
            hd = _hd_orig(c, p)
            fw2p[:, c * 128 + p] = fw2[hd, :]
            b3l[c, p] = fb2[hd]
    fill(wc0, "fw2p", fw2p)
    fill(wc0, "b3l", b3l)
    b3r = np.zeros((4, 4 * BS), f)
    for c in range(4):
        b3r[c, c * BS : (c + 1) * BS] = 1.0
    fill(wc0, "b3r", b3r)
    gneg = np.zeros((128, 3 * 4 * H), f)
    for ai, aval in enumerate(_AVALS):
        for c in range(4):
            for p in range(128):
                h = 16 * c + (p % 16)
                gneg[p, (ai * 4 + c) * H + h] = -2.0 * aval
    fill(wc0, "gneg", gneg)
    onesa = np.zeros((D, 3 * H), f)
    for ai, aval in enumerate(_AVALS):
        onesa[:, ai * H : (ai + 1) * H] = aval
    fill(wc0, "onesa", onesa)
    # ww[p, (v*4+c)*W + w] = -2 a_v * fw0p[h(p,c), w], h(p,c) = 16c + p%16;
    # fsa[d, v*W + w] = a_v * sum_h fw0p[h, w]  (fold fw0 @ k~ into PSUM)
    fw0p_arr = np.asarray(fw0, f).T  # [H, W]
    fsum = fw0p_arr.sum(axis=0)  # [W]
    ww = np.zeros((128, 2 * 4 * W), f)
    pidx = np.arange(128)
    for v in range(2):
        for c in range(4):
            hrows = 16 * c + (pidx % 16)
            ww[:, (v * 4 + c) * W : (v * 4 + c + 1) * W] = (
                -2.0 * _AVALS[v] * fw0p_arr[hrows, :]
            )
    fill(wc0, "ww", ww)
    fsa = np.zeros((D, 2 * W), f)
    for v in range(2):
        fsa[:, v * W : (v + 1) * W] = _AVALS[v] * fsum[None, :]
    fill(wc0, "fsa", fsa)
    fill(wc0, "iw0p", np.ascontiguousarray(np.asarray(iw0, f).T))
    fill(wc0, "iw1p", np.ascontiguousarray(np.asarray(iw1, f).T))
    fill(wc0, "iw2p", np.ascontiguousarray(np.asarray(iw2, f).T))
    fill(wc0, "lwT", np.ascontiguousarray(np.asarray(lw, f).reshape(1, H).T))
    fill(wc0, "ib0", np.asarray(ib0, f)[:, None])
    fill(wc0, "ib1", np.asarray(ib1, f)[:, None])
    fill(wc0, "ib2", np.asarray(ib2, f)[:, None])
    fill(wc0, "fb0", np.asarray(fb0, f)[:, None])
    fill(wc0, "fb1", np.asarray(fb1, f)[:, None])
    fill(wc0, "lbneg", -np.asarray(lb, f).reshape(1, 1))

    in_maps = []
    for i in range(NCORES):
        sl = slice(i * BS, (i + 1) * BS)
        wc = wc0.copy()
        fill(wc, "x0T", np.ascontiguousarray(ca[sl, 0, :].T))
        m = {"wconst": wc}
        for name, arr in (("dxt0", dX1), ("dxt1", dX23), ("dxt2", dX4)):
            m[name] = np.ascontiguousarray(
                arr[sl].transpose(2, 1, 0).reshape(D, -1)
            )
        in_maps.append(m)
    return in_maps


_CACHE = {}


def _get_nc(nsteps):
    if nsteps not in _CACHE:
        _CACHE[nsteps] = build_bass(nsteps)
    return _CACHE[nsteps]


def kernel(**inputs):
    from concourse.bass_utils import run_bass_kernel_spmd

    nsteps = NSTEPS_FULL
    in_maps = prep_inputs(nsteps=nsteps, **inputs)
    nc = _get_nc(nsteps)
    res = run_bass_kernel_spmd(nc, in_maps, list(range(NCORES)))
    outs = [res.results[i]["out"].reshape(BS) for i in range(NCORES)]
    return np.concatenate(outs, axis=0).astype(np.float32)
